# revision 27
# baseline (speedup 1.0000x reference)
"""CharDecoder LSTM (T=21, B=4096, H=1024, E=50, V=96) on 8 trn2 NeuronCores.

Strategy: data-parallel over batch (512 rows/core), everything feature-major
("transposed") on device so the recurrence h_t -> h_{t+1} needs no transposes:

  gatesT (4H, B) = W_hh @ hT  accumulated in PSUM over 8 K-tiles of H,
  plus a 9th matmul  emb_proj @ onehotT  (K padded 96->128; zero rows) that
  adds the input projection x_proj (embedding gather == one-hot matmul;
  emb_proj = emb@W_ih.T precomputed on device once; one-hots built on device
  from ids via is_equal).  b_ih+b_hh are folded into the gate activations
  (ACT computes func(in + bias) natively).  h is double-buffered across
  steps; c updates in place; W_out is M-padded to 128.

  Matmuls run in float32r (fp32 with round-nearest-12-mantissa-bits operand
  ingestion) at full 1 row/cycle PE rate.  c stays fp32.

Self-contained: hardcodes shapes; host side only reshapes/transposes/shards.
"""

import numpy as np
from contextlib import ExitStack

T = 21
B_FULL = 4096
H = 1024
E = 50
V = 96
NCORES = 8
B = B_FULL // NCORES  # 512
G = 4 * H  # 4096
KT = H // 128  # 8 k-tiles over H
NGT = G // 128  # 32 gate-feature tiles

_cache = {}


def _build(n_steps=T):
    import concourse.bacc as bacc
    import concourse.mybir as mybir
    import concourse.tile as tile

    dt = mybir.dt
    AF = mybir.ActivationFunctionType
    ALU = mybir.AluOpType

    nc = bacc.Bacc("TRN2", target_bir_lowering=False, num_devices=NCORES)

    ids_d = nc.dram_tensor("ids_f", [n_steps, B], dt.float32r, kind="ExternalInput")
    h0_d = nc.dram_tensor("h0T", [H, B], dt.float32r, kind="ExternalInput")
    c0_d = nc.dram_tensor("c0T", [H, B], dt.float32, kind="ExternalInput")
    whh_d = nc.dram_tensor("whhT", [H, G], dt.float32r, kind="ExternalInput")
    wih_d = nc.dram_tensor("wihT", [E, G], dt.float32, kind="ExternalInput")
    emb_d = nc.dram_tensor("embT", [E, V], dt.float32, kind="ExternalInput")
    bias_d = nc.dram_tensor("bias_pm", [128, NGT], dt.float32, kind="ExternalInput")
    wout_d = nc.dram_tensor("woutT", [H, 128], dt.float32r, kind="ExternalInput")
    bout_d = nc.dram_tensor("bout", [V, 1], dt.float32, kind="ExternalInput")
    iota_d = nc.dram_tensor("iota_v", [128, 1], dt.float32, kind="ExternalInput")

    sco_d = nc.dram_tensor("scoresT", [n_steps, V, B], dt.float32, kind="ExternalOutput")
    hT_d = nc.dram_tensor("hT_out", [H, B], dt.float32, kind="ExternalOutput")
    cT_d = nc.dram_tensor("cT_out", [H, B], dt.float32, kind="ExternalOutput")

    with tile.TileContext(nc) as tc, ExitStack() as ctx:
        consts = ctx.enter_context(tc.tile_pool(name="consts", bufs=1))

        w_sb = consts.tile([128, KT * G], dt.float32r, tag="whh")
        wout_sb = consts.tile([128, KT * 128], dt.float32r, tag="wout")
        embp_sb = consts.tile([128, G], dt.float32r, tag="embp")
        bias_sb = consts.tile([128, NGT], dt.float32, tag="bias")
        bout_sb = consts.tile([V, 1], dt.float32, tag="bout")
        iota_sb = consts.tile([128, 1], dt.float32, tag="iota")
        h_bufs = [
            consts.tile([128, KT * B], dt.float32r, tag="h_even", name="h_even"),
            consts.tile([128, KT * B], dt.float32r, tag="h_odd", name="h_odd"),
        ]
        c_sb = consts.tile([128, KT * B], dt.float32, tag="c")

        nc.gpsimd.memset(embp_sb[:].bitcast(dt.float32), 0.0)
        nc.sync.dma_start(bias_sb[:], bias_d[:, :])
        nc.sync.dma_start(bout_sb[:], bout_d[:, :])
        nc.sync.dma_start(iota_sb[:], iota_d[:, :])
        nc.sync.dma_start(
            wout_sb[:].rearrange("p (k v) -> p k v", k=KT),
            wout_d.rearrange("(k p) v -> p k v", p=128),
        )  # padded to M=128 (rows 96..127 zero) to avoid col-group reconfig
        # Bulk loads go on the SWDGE (gpsimd) queue in consumption order:
        # h0/c0 first (step 0 needs them immediately), then W_hh per k-tile
        # so step-0 k-outer matmuls start as each tile lands. The sync queue
        # stays free for the small per-step ids DMAs.
        nc.gpsimd.dma_start(
            h_bufs[0][:].rearrange("p (k b) -> p k b", k=KT),
            h0_d.rearrange("(k p) b -> p k b", p=128),
        )
        nc.gpsimd.dma_start(
            c_sb[:].rearrange("p (k b) -> p k b", k=KT),
            c0_d.rearrange("(k p) b -> p k b", p=128),
        )
        for k in range(KT):
            nc.gpsimd.dma_start(
                w_sb[:, k * G : (k + 1) * G],
                whh_d[k * 128 : (k + 1) * 128, :],
            )

        # ---- emb_proj = emb @ W_ih.T, layout [V, G], computed once in fp32
        with (
            tc.tile_pool(name="pre", bufs=1) as pre,
            tc.tile_pool(name="preps", bufs=2, space="PSUM") as preps,
        ):
            embT_sb = pre.tile([E, V], dt.float32, tag="embT")
            nc.sync.dma_start(embT_sb[:], emb_d[:, :])
            for ch in range(G // 512):
                wih_ch = pre.tile([E, 512], dt.float32, tag="wih", bufs=4)
                nc.sync.dma_start(wih_ch[:], wih_d[:, ch * 512 : (ch + 1) * 512])
                pp = preps.tile([V, 512], dt.float32, tag="pp")
                nc.tensor.matmul(pp[:], embT_sb[:], wih_ch[:], start=True, stop=True)
                nc.vector.tensor_copy(embp_sb[0:V, ch * 512 : (ch + 1) * 512], pp[:])

        oh_p = ctx.enter_context(tc.tile_pool(name="oh", bufs=2))
        sig_p = ctx.enter_context(tc.tile_pool(name="sig", bufs=1))
        thc_p = ctx.enter_context(tc.tile_pool(name="thc", bufs=1))
        sco_p = ctx.enter_context(tc.tile_pool(name="sco", bufs=1))
        gps = ctx.enter_context(tc.tile_pool(name="gps", bufs=7, space="PSUM"))
        scps = ctx.enter_context(tc.tile_pool(name="scps", bufs=1, space="PSUM"))

        for t in range(n_steps):
            h_prev = h_bufs[t % 2]
            h_next = h_bufs[(t + 1) % 2]

            oh_t = oh_p.tile([128, B], dt.float32r, tag="oh")
            nc.sync.dma_start(oh_t[:], ids_d[t : t + 1, :].broadcast_to((128, B)))
            nc.vector.tensor_scalar(
                oh_t[:], oh_t[:].bitcast(dt.float32), iota_sb[:], None,
                op0=ALU.is_equal,
            )

            for hm in range(KT):
                pg = [gps.tile([128, B], dt.float32, tag="g", name=f"g{t}_{hm}_{gi}") for gi in range(4)]
                if t == 0:
                    # k-outer: overlap step-0 gates with the streaming W load
                    # (gate matmuls for k-tile k start as soon as it lands).
                    for k in range(KT):
                        for gi in range(4):
                            m = gi * 8 + hm
                            nc.tensor.matmul(
                                pg[gi][:],
                                w_sb[:, k * G + m * 128 : k * G + (m + 1) * 128],
                                h_prev[:, k * B : (k + 1) * B],
                                start=(k == 0),
                                stop=False,
                            )
                    for gi in range(4):
                        m = gi * 8 + hm
                        nc.tensor.matmul(
                            pg[gi][:],
                            embp_sb[:, m * 128 : (m + 1) * 128],
                            oh_t[:],
                            start=False,
                            stop=True,
                        )
                else:
                    for gi in range(4):
                        m = gi * 8 + hm
                        pp = pg[gi]
                        for k in range(KT):
                            nc.tensor.matmul(
                                pp[:],
                                w_sb[:, k * G + m * 128 : k * G + (m + 1) * 128],
                                h_prev[:, k * B : (k + 1) * B],
                                start=(k == 0),
                                stop=False,
                            )
                        nc.tensor.matmul(
                            pp[:],
                            embp_sb[:, m * 128 : (m + 1) * 128],
                            oh_t[:],
                            start=False,
                            stop=True,
                        )
                pi, pf, pgg, po = pg
                mi, mf, mg, mo = hm, 8 + hm, 16 + hm, 24 + hm

                sig_i = sig_p.tile([128, B], dt.float32, tag="sigi")
                nc.scalar.activation(
                    sig_i[:], pi[:], AF.Sigmoid, bias=bias_sb[:, mi : mi + 1]
                )
                nc.scalar.activation(
                    pf[:], pf[:], AF.Sigmoid, bias=bias_sb[:, mf : mf + 1]
                )
                nc.scalar.activation(
                    pgg[:], pgg[:], AF.Tanh, bias=bias_sb[:, mg : mg + 1]
                )
                nc.scalar.activation(
                    po[:], po[:], AF.Sigmoid, bias=bias_sb[:, mo : mo + 1]
                )

                nc.vector.tensor_mul(pgg[:], sig_i[:], pgg[:])  # pgg = i*g in PSUM
                c_sl = c_sb[:, hm * B : (hm + 1) * B]
                nc.vector.tensor_mul(c_sl, pf[:], c_sl)
                nc.vector.tensor_add(c_sl, c_sl, pgg[:])
                thc = thc_p.tile([128, B], dt.float32, tag="thc")
                nc.scalar.activation(thc[:], c_sl, AF.Tanh)
                h_sl = h_next[:, hm * B : (hm + 1) * B]
                nc.vector.tensor_mul(h_sl, po[:], thc[:])
                if t == n_steps - 1:
                    # drain final states while the rest of the step computes
                    nc.sync.dma_start(
                        hT_d[hm * 128 : (hm + 1) * 128, :],
                        h_sl.bitcast(dt.float32),
                    )
                    nc.sync.dma_start(cT_d[hm * 128 : (hm + 1) * 128, :], c_sl)

            sc = scps.tile([128, B], dt.float32, tag="sc")
            for k in range(KT):
                nc.tensor.matmul(
                    sc[:],
                    wout_sb[:, k * 128 : (k + 1) * 128],
                    h_next[:, k * B : (k + 1) * B],
                    start=(k == 0),
                    stop=(k == KT - 1),
                )
            so = sco_p.tile([V, B], dt.float32, tag="so")
            nc.scalar.activation(so[:], sc[0:V, :], AF.Identity, bias=bout_sb[:])
            nc.sync.dma_start(sco_d[t], so[:])



    nc.compile()
    return nc


def _prep_in_maps(input_ids, h0, c0, emb, W_ih, W_hh, b_ih, b_hh, W_out, b_out):
    ids = np.asarray(input_ids)
    h0 = np.asarray(h0, dtype=np.float32)
    c0 = np.asarray(c0, dtype=np.float32)
    emb = np.asarray(emb, dtype=np.float32)
    W_ih = np.asarray(W_ih, dtype=np.float32)
    W_hh = np.asarray(W_hh, dtype=np.float32)
    b_ih = np.asarray(b_ih, dtype=np.float32)
    b_hh = np.asarray(b_hh, dtype=np.float32)
    W_out = np.asarray(W_out, dtype=np.float32)
    b_out = np.asarray(b_out, dtype=np.float32)

    ids_f = ids.astype(np.float32)  # values 0..95, exact in fp32
    whhT = np.ascontiguousarray(W_hh.T)  # (H, G)
    wihT = np.ascontiguousarray(W_ih.T)  # (E, G)
    embT = np.ascontiguousarray(emb.T)  # (E, V)
    bias_pm = np.ascontiguousarray((b_ih + b_hh).reshape(NGT, 128).T)  # (128, NGT)
    woutT = np.zeros((H, 128), dtype=np.float32)
    woutT[:, :V] = W_out.T
    boutc = np.ascontiguousarray(b_out.reshape(V, 1))
    iota = np.full((128, 1), 1000.0, dtype=np.float32)
    iota[:V, 0] = np.arange(V, dtype=np.float32)

    in_maps = []
    for c in range(NCORES):
        sl = slice(c * B, (c + 1) * B)
        in_maps.append(
            {
                "ids_f": np.ascontiguousarray(ids_f[:, sl]),
                "h0T": np.ascontiguousarray(h0[0, sl, :].T),
                "c0T": np.ascontiguousarray(c0[0, sl, :].T),
                "whhT": whhT,
                "wihT": wihT,
                "embT": embT,
                "bias_pm": bias_pm,
                "woutT": woutT,
                "bout": boutc,
                "iota_v": iota,
            }
        )
    return in_maps


def _assemble(results):
    scores = np.concatenate(
        [np.transpose(r["scoresT"], (0, 2, 1)) for r in results], axis=1
    )  # (T, B_FULL, V)
    h_T = np.concatenate([r["hT_out"].T for r in results], axis=0)[None]  # (1,B,H)
    c_T = np.concatenate([r["cT_out"].T for r in results], axis=0)[None]
    return scores, (h_T, c_T)


def _run(in_maps, trace=False, **kw):
    from concourse.bass_utils import run_bass_kernel_spmd

    if "nc" not in _cache:
        _cache["nc"] = _build()
    return run_bass_kernel_spmd(
        _cache["nc"], in_maps, core_ids=list(range(NCORES)), trace=trace, **kw
    )


def kernel(input_ids, h0, c0, emb, W_ih, W_hh, b_ih, b_hh, W_out, b_out):
    in_maps = _prep_in_maps(
        input_ids, h0, c0, emb, W_ih, W_hh, b_ih, b_hh, W_out, b_out
    )
    res = _run(in_maps, trace=False)
    return _assemble(res.results)


# revision 32
# speedup vs baseline: 1.0034x; 1.0034x over previous
"""CharDecoder LSTM (T=21, B=4096, H=1024, E=50, V=96) on 8 trn2 NeuronCores.

Strategy: data-parallel over batch (512 rows/core), everything feature-major
("transposed") on device so the recurrence h_t -> h_{t+1} needs no transposes:

  gatesT (4H, B) = W_hh @ hT  accumulated in PSUM over 8 K-tiles of H,
  plus a 9th matmul  emb_proj @ onehotT  (K padded 96->128; zero rows) that
  adds the input projection x_proj (embedding gather == one-hot matmul;
  emb_proj = emb@W_ih.T precomputed on device once; one-hots built on device
  from ids via is_equal).  b_ih+b_hh are folded into the gate activations
  (ACT computes func(in + bias) natively).  h is double-buffered across
  steps; c updates in place; W_out is M-padded to 128.

  Matmuls run in float32r (fp32 with round-nearest-12-mantissa-bits operand
  ingestion) at full 1 row/cycle PE rate.  c stays fp32.

Self-contained: hardcodes shapes; host side only reshapes/transposes/shards.
"""

import numpy as np
from contextlib import ExitStack

T = 21
B_FULL = 4096
H = 1024
E = 50
V = 96
NCORES = 8
B = B_FULL // NCORES  # 512
G = 4 * H  # 4096
KT = H // 128  # 8 k-tiles over H
NGT = G // 128  # 32 gate-feature tiles

_cache = {}


def _build(n_steps=T):
    import concourse.bacc as bacc
    import concourse.mybir as mybir
    import concourse.tile as tile

    dt = mybir.dt
    AF = mybir.ActivationFunctionType
    ALU = mybir.AluOpType

    nc = bacc.Bacc("TRN2", target_bir_lowering=False, num_devices=NCORES)

    ids_d = nc.dram_tensor("ids_f", [n_steps, B], dt.float32r, kind="ExternalInput")
    h0_d = nc.dram_tensor("h0T", [H, B], dt.float32r, kind="ExternalInput")
    c0_d = nc.dram_tensor("c0T", [H, B], dt.float32, kind="ExternalInput")
    whh_d = nc.dram_tensor("whhT", [H, G], dt.float32r, kind="ExternalInput")
    wih_d = nc.dram_tensor("wihT", [E, G], dt.float32, kind="ExternalInput")
    emb_d = nc.dram_tensor("embT", [E, V], dt.float32, kind="ExternalInput")
    bias_d = nc.dram_tensor("bias_pm", [128, NGT], dt.float32, kind="ExternalInput")
    wout_d = nc.dram_tensor("woutT", [H, 128], dt.float32r, kind="ExternalInput")
    bout_d = nc.dram_tensor("bout", [V, 1], dt.float32, kind="ExternalInput")
    iota_d = nc.dram_tensor("iota_v", [128, 1], dt.float32, kind="ExternalInput")

    sco_d = nc.dram_tensor("scoresT", [n_steps, V, B], dt.float32, kind="ExternalOutput")
    hT_d = nc.dram_tensor("hT_out", [H, B], dt.float32, kind="ExternalOutput")
    cT_d = nc.dram_tensor("cT_out", [H, B], dt.float32, kind="ExternalOutput")

    with tile.TileContext(nc) as tc, ExitStack() as ctx:
        consts = ctx.enter_context(tc.tile_pool(name="consts", bufs=1))

        w_sb = consts.tile([128, KT * G], dt.float32r, tag="whh")
        wout_sb = consts.tile([128, KT * 128], dt.float32r, tag="wout")
        embp_sb = consts.tile([128, G], dt.float32r, tag="embp")
        bias_sb = consts.tile([128, NGT], dt.float32, tag="bias")
        bout_sb = consts.tile([V, 1], dt.float32, tag="bout")
        iota_sb = consts.tile([128, 1], dt.float32, tag="iota")
        h_bufs = [
            consts.tile([128, KT * B], dt.float32r, tag="h_even", name="h_even"),
            consts.tile([128, KT * B], dt.float32r, tag="h_odd", name="h_odd"),
        ]
        c_sb = consts.tile([128, KT * B], dt.float32, tag="c")

        nc.gpsimd.memset(embp_sb[:].bitcast(dt.float32), 0.0)
        nc.sync.dma_start(bias_sb[:], bias_d[:, :])
        nc.sync.dma_start(bout_sb[:], bout_d[:, :])
        nc.sync.dma_start(iota_sb[:], iota_d[:, :])
        nc.sync.dma_start(
            wout_sb[:].rearrange("p (k v) -> p k v", k=KT),
            wout_d.rearrange("(k p) v -> p k v", p=128),
        )  # padded to M=128 (rows 96..127 zero) to avoid col-group reconfig
        # Bulk loads go on the SWDGE (gpsimd) queue in consumption order:
        # h0/c0 first (step 0 needs them immediately), then W_hh per k-tile
        # so step-0 k-outer matmuls start as each tile lands. The sync queue
        # stays free for the small per-step ids DMAs.
        nc.gpsimd.dma_start(
            h_bufs[0][:].rearrange("p (k b) -> p k b", k=KT),
            h0_d.rearrange("(k p) b -> p k b", p=128),
        )
        nc.gpsimd.dma_start(
            c_sb[:].rearrange("p (k b) -> p k b", k=KT),
            c0_d.rearrange("(k p) b -> p k b", p=128),
        )
        for k in range(KT):
            nc.gpsimd.dma_start(
                w_sb[:, k * G : (k + 1) * G],
                whh_d[k * 128 : (k + 1) * 128, :],
            )

        # ---- emb_proj = emb @ W_ih.T, layout [V, G], computed once in fp32
        with (
            tc.tile_pool(name="pre", bufs=1) as pre,
            tc.tile_pool(name="preps", bufs=2, space="PSUM") as preps,
        ):
            embT_sb = pre.tile([E, V], dt.float32, tag="embT")
            nc.sync.dma_start(embT_sb[:], emb_d[:, :])
            for ch in range(G // 512):
                wih_ch = pre.tile([E, 512], dt.float32, tag="wih", bufs=4)
                nc.sync.dma_start(wih_ch[:], wih_d[:, ch * 512 : (ch + 1) * 512])
                pp = preps.tile([V, 512], dt.float32, tag="pp")
                nc.tensor.matmul(pp[:], embT_sb[:], wih_ch[:], start=True, stop=True)
                nc.vector.tensor_copy(embp_sb[0:V, ch * 512 : (ch + 1) * 512], pp[:])

        oh_p = ctx.enter_context(tc.tile_pool(name="oh", bufs=2))
        sig_p = ctx.enter_context(tc.tile_pool(name="sig", bufs=1))
        thc_p = ctx.enter_context(tc.tile_pool(name="thc", bufs=1))
        sco_p = ctx.enter_context(tc.tile_pool(name="sco", bufs=1))
        gps = ctx.enter_context(tc.tile_pool(name="gps", bufs=8, space="PSUM"))

        def emit_cell_update(t, hm, pg, h_next):
            pi, pf, pgg, po = pg
            mi, mf, mg, mo = hm, 8 + hm, 16 + hm, 24 + hm
            sig_i = sig_p.tile([128, B], dt.float32, tag="sigi", name=f"si{t}_{hm}")
            nc.scalar.activation(
                sig_i[:], pi[:], AF.Sigmoid, bias=bias_sb[:, mi : mi + 1]
            )
            nc.scalar.activation(pf[:], pf[:], AF.Sigmoid, bias=bias_sb[:, mf : mf + 1])
            nc.scalar.activation(pgg[:], pgg[:], AF.Tanh, bias=bias_sb[:, mg : mg + 1])
            nc.scalar.activation(po[:], po[:], AF.Sigmoid, bias=bias_sb[:, mo : mo + 1])
            nc.vector.tensor_mul(pgg[:], sig_i[:], pgg[:])  # pgg = i*g in PSUM
            c_sl = c_sb[:, hm * B : (hm + 1) * B]
            nc.vector.tensor_mul(c_sl, pf[:], c_sl)
            nc.vector.tensor_add(c_sl, c_sl, pgg[:])
            thc = thc_p.tile([128, B], dt.float32, tag="thc", name=f"th{t}_{hm}")
            nc.scalar.activation(thc[:], c_sl, AF.Tanh)
            h_sl = h_next[:, hm * B : (hm + 1) * B]
            nc.vector.tensor_mul(h_sl, po[:], thc[:])
            if t == n_steps - 1:
                # drain final states while the rest of the step computes
                nc.sync.dma_start(
                    hT_d[hm * 128 : (hm + 1) * 128, :], h_sl.bitcast(dt.float32)
                )
                nc.sync.dma_start(cT_d[hm * 128 : (hm + 1) * 128, :], c_sl)

        for t in range(n_steps):
            h_prev = h_bufs[t % 2]
            h_next = h_bufs[(t + 1) % 2]

            oh_t = oh_p.tile([128, B], dt.float32r, tag="oh")
            nc.sync.dma_start(oh_t[:], ids_d[t : t + 1, :].broadcast_to((128, B)))
            nc.vector.tensor_scalar(
                oh_t[:], oh_t[:].bitcast(dt.float32), iota_sb[:], None,
                op0=ALU.is_equal,
            )

            def emit_gate_matmuls(hm_list, pg_by_hm):
                if t == 0:
                    # one-hot matmuls first: they only need oh_t (ready ~10us)
                    # so PE has work before the first W k-tile lands; then
                    # k-outer so gates consume the W stream as it arrives.
                    for hm in hm_list:
                        for gi in range(4):
                            m = gi * 8 + hm
                            nc.tensor.matmul(
                                pg_by_hm[hm][gi][:],
                                embp_sb[:, m * 128 : (m + 1) * 128],
                                oh_t[:],
                                start=True,
                                stop=False,
                            )
                    for k in range(KT):
                        for hm in hm_list:
                            for gi in range(4):
                                m = gi * 8 + hm
                                nc.tensor.matmul(
                                    pg_by_hm[hm][gi][:],
                                    w_sb[:, k * G + m * 128 : k * G + (m + 1) * 128],
                                    h_prev[:, k * B : (k + 1) * B],
                                    start=False,
                                    stop=(k == KT - 1),
                                )
                else:
                    for hm in hm_list:
                        for gi in range(4):
                            m = gi * 8 + hm
                            pp = pg_by_hm[hm][gi]
                            for k in range(KT):
                                nc.tensor.matmul(
                                    pp[:],
                                    w_sb[:, k * G + m * 128 : k * G + (m + 1) * 128],
                                    h_prev[:, k * B : (k + 1) * B],
                                    start=(k == 0),
                                    stop=False,
                                )
                            nc.tensor.matmul(
                                pp[:],
                                embp_sb[:, m * 128 : (m + 1) * 128],
                                oh_t[:],
                                start=False,
                                stop=True,
                            )

            # Step 0 runs hm-PAIR waves (8 PSUM banks) so the k-outer order
            # consumes each arriving W k-tile with 8 matmuls; later steps run
            # per-hm (4 banks) for the deepest PE-ahead pipeline.
            wave = 1
            for hw0 in range(0, KT, wave):
                hms = list(range(hw0, hw0 + wave))
                pg_by_hm = {
                    hm: [
                        gps.tile([128, B], dt.float32, tag="g", name=f"g{t}_{hm}_{gi}")
                        for gi in range(4)
                    ]
                    for hm in hms
                }
                emit_gate_matmuls(hms, pg_by_hm)
                for hm in hms:
                    emit_cell_update(t, hm, pg_by_hm[hm], h_next)

            sc = gps.tile([128, B], dt.float32, tag="g", name=f"sc{t}")
            for k in range(KT):
                nc.tensor.matmul(
                    sc[:],
                    wout_sb[:, k * 128 : (k + 1) * 128],
                    h_next[:, k * B : (k + 1) * B],
                    start=(k == 0),
                    stop=(k == KT - 1),
                )
            so = sco_p.tile([V, B], dt.float32, tag="so")
            nc.scalar.activation(so[:], sc[0:V, :], AF.Identity, bias=bout_sb[:])
            nc.sync.dma_start(sco_d[t], so[:])



    nc.compile()
    return nc


def _prep_in_maps(input_ids, h0, c0, emb, W_ih, W_hh, b_ih, b_hh, W_out, b_out):
    ids = np.asarray(input_ids)
    h0 = np.asarray(h0, dtype=np.float32)
    c0 = np.asarray(c0, dtype=np.float32)
    emb = np.asarray(emb, dtype=np.float32)
    W_ih = np.asarray(W_ih, dtype=np.float32)
    W_hh = np.asarray(W_hh, dtype=np.float32)
    b_ih = np.asarray(b_ih, dtype=np.float32)
    b_hh = np.asarray(b_hh, dtype=np.float32)
    W_out = np.asarray(W_out, dtype=np.float32)
    b_out = np.asarray(b_out, dtype=np.float32)

    ids_f = ids.astype(np.float32)  # values 0..95, exact in fp32
    whhT = np.ascontiguousarray(W_hh.T)  # (H, G)
    wihT = np.ascontiguousarray(W_ih.T)  # (E, G)
    embT = np.ascontiguousarray(emb.T)  # (E, V)
    bias_pm = np.ascontiguousarray((b_ih + b_hh).reshape(NGT, 128).T)  # (128, NGT)
    woutT = np.zeros((H, 128), dtype=np.float32)
    woutT[:, :V] = W_out.T
    boutc = np.ascontiguousarray(b_out.reshape(V, 1))
    iota = np.full((128, 1), 1000.0, dtype=np.float32)
    iota[:V, 0] = np.arange(V, dtype=np.float32)

    in_maps = []
    for c in range(NCORES):
        sl = slice(c * B, (c + 1) * B)
        in_maps.append(
            {
                "ids_f": np.ascontiguousarray(ids_f[:, sl]),
                "h0T": np.ascontiguousarray(h0[0, sl, :].T),
                "c0T": np.ascontiguousarray(c0[0, sl, :].T),
                "whhT": whhT,
                "wihT": wihT,
                "embT": embT,
                "bias_pm": bias_pm,
                "woutT": woutT,
                "bout": boutc,
                "iota_v": iota,
            }
        )
    return in_maps


def _assemble(results):
    scores = np.concatenate(
        [np.transpose(r["scoresT"], (0, 2, 1)) for r in results], axis=1
    )  # (T, B_FULL, V)
    h_T = np.concatenate([r["hT_out"].T for r in results], axis=0)[None]  # (1,B,H)
    c_T = np.concatenate([r["cT_out"].T for r in results], axis=0)[None]
    return scores, (h_T, c_T)


def _run(in_maps, trace=False, **kw):
    from concourse.bass_utils import run_bass_kernel_spmd

    if "nc" not in _cache:
        _cache["nc"] = _build()
    return run_bass_kernel_spmd(
        _cache["nc"], in_maps, core_ids=list(range(NCORES)), trace=trace, **kw
    )


def kernel(input_ids, h0, c0, emb, W_ih, W_hh, b_ih, b_hh, W_out, b_out):
    in_maps = _prep_in_maps(
        input_ids, h0, c0, emb, W_ih, W_hh, b_ih, b_hh, W_out, b_out
    )
    res = _run(in_maps, trace=False)
    return _assemble(res.results)
